# revision 12
# baseline (speedup 1.0000x reference)
"""Windowed attention block (LeViT-style) on 8 TRN2 NeuronCores via Bass/Tile.

LayerNorm -> QKV -> per-head biased softmax attention -> output projection,
B=256 windows, N=196 tokens, DIM=384, 12 heads of dim 32.

Strategy: data-parallel over windows (32 per core), parameters replicated.
All matmuls in bf16 on the TensorEngine; softmax exp on ScalarE reading
scores straight out of PSUM; relative-position bias applied multiplicatively
(exp(s+b) = exp(s)*exp(b), exp(b) precomputed); softmax denominator comes
free from a ones-column appended to V; bias-mult on GPSIMD; transposes on
the DMA xbar (dense outputs only - strided xbar writes are broken on HW);
all matmul operands at partition base 0 (nonzero-base/tile_position packing
is broken on this stack), with a small SBUF->SBUF DMA remap giving per-head
base-0 q/k tiles.

Host-side prep: layout/dtype transforms (bf16 cast, weight reorder/transpose,
bias gather+exp) plus the input LayerNorm fold (stats+normalize+transpose),
which is <0.3% of the FLOPs.
"""

import numpy as np
import ml_dtypes

import concourse.bass as bass
import concourse.bacc as bacc
import concourse.tile as tile
from concourse import mybir
from concourse.bass_utils import run_bass_kernel_spmd

BF16 = mybir.dt.bfloat16
F32 = mybir.dt.float32
AF = mybir.ActivationFunctionType
OP = mybir.AluOpType
bf16 = ml_dtypes.bfloat16

B, N, DIM = 256, 196, 384
H, KD, VD = 12, 32, 32
EPS = 1e-5
NCORES = 8
WPC = B // NCORES  # windows per core
SCALE = KD ** -0.5

_CACHE = {}


def _mkap(t, extra_off, dims):
    """Manual AP: keep t's partition dim, custom free dims, +offset (elems)."""
    return bass.AP(
        tensor=t.tensor,
        offset=t.offset + extra_off,
        ap=[list(t.ap[0])] + [list(d) for d in dims],
    )


def _build_nc(nwin=WPC):
    nc = bacc.Bacc("TRN2", target_bir_lowering=False, debug=False)

    # xn: normalized input, pre-transposed to feature-major blocks.
    # [128, nwin, 6, 128]: window w, (s, kt) combined; element [dp, w, 3s+kt, t]
    # = xn[w, 128*s + t, 128*kt + dp]  (s=1 rows 68-127 are zero padding).
    xn_d = nc.dram_tensor("xn", [128, nwin, 6, 128], BF16, kind="ExternalInput")
    wqk_d = nc.dram_tensor("wqk", [128, 3, 768], BF16, kind="ExternalInput")
    wv_d = nc.dram_tensor("wv", [128, 3, 384], BF16, kind="ExternalInput")
    pw_d = nc.dram_tensor("pw", [128, 3, 384], BF16, kind="ExternalInput")
    eb_d = nc.dram_tensor("expb", [N, H, N], BF16, kind="ExternalInput")
    o_d = nc.dram_tensor("out", [nwin, N, DIM], BF16, kind="ExternalOutput")

    from contextlib import ExitStack

    with tile.TileContext(nc) as tc, ExitStack() as ctx:
        consts = ctx.enter_context(tc.tile_pool(name="consts", bufs=1))
        wqk = consts.tile([128, 3, 768], BF16, tag="wqk")
        nc.sync.dma_start(out=wqk, in_=wqk_d[:, :, :])
        wv = consts.tile([128, 3, 384], BF16, tag="wv")
        nc.sync.dma_start(out=wv, in_=wv_d[:, :, :])
        pw = consts.tile([128, 3, 384], BF16, tag="pw")
        nc.sync.dma_start(out=pw, in_=pw_d[:, :, :])
        eb0 = consts.tile([128, H, N], BF16, tag="eb0")
        nc.sync.dma_start(out=eb0, in_=eb_d[0:128, :, :])
        eb1 = consts.tile([68, H, N], BF16, tag="eb1")
        nc.sync.dma_start(out=eb1, in_=eb_d[128:196, :, :])

        # avU staging tiles live for the whole kernel so the pad region
        # (transposed but never consumed) is memset only once.
        avUs = [consts.tile([112, 6, 256], BF16, tag=f"avU{i}", name=f"avU{i}")
                for i in range(2)]
        for t in avUs:
            nc.vector.memset(t[:, :, :], 0.0)

        xnt = ctx.enter_context(tc.tile_pool(name="xnt", bufs=3))
        qkt = ctx.enter_context(tc.tile_pool(name="qkt", bufs=2))
        qk32 = ctx.enter_context(tc.tile_pool(name="qk32", bufs=2))
        vap = ctx.enter_context(tc.tile_pool(name="vap", bufs=2))
        eup = ctx.enter_context(tc.tile_pool(name="eup", bufs=2))
        pbp = ctx.enter_context(tc.tile_pool(name="pbp", bufs=2))
        oaup = ctx.enter_context(tc.tile_pool(name="oaup", bufs=2))
        oap = ctx.enter_context(tc.tile_pool(name="oap", bufs=2))
        otp = ctx.enter_context(tc.tile_pool(name="otp", bufs=2))
        obp = ctx.enter_context(tc.tile_pool(name="obp", bufs=4))
        rcp = ctx.enter_context(tc.tile_pool(name="rcp", bufs=2))

        pssp = ctx.enter_context(tc.tile_pool(name="pssp", bufs=1, space="PSUM"))
        psm = ctx.enter_context(tc.tile_pool(name="psm", bufs=3, space="PSUM"))

        for p in range(nwin // 2):
            # xnT per pair: [128, 4 halves (w0s0 w0s1 w1s0 w1s1), 3 kt, 128]
            xnT = xnt.tile([128, 4, 3, 128], BF16, tag="xnT")
            nc.sync.dma_start(
                out=xnT,
                in_=xn_d[:, 2 * p:2 * p + 2, :, :].rearrange(
                    "d w (s kt) t -> d (w s) kt t", s=2))

            qkT = qkt.tile([128, 6, 392], BF16, tag="qkT")
            # QK projection: rhs streams token halves of both windows.
            # Each psum col-region completes its kt accumulation before the
            # next region starts (start=True marks whole-bank zero regions).
            for fc in range(6):
                pq = psm.tile([128, 512], F32, tag="ps")
                for hoff, tw, c0, c1 in ((0, 128, 0, 256), (384, 68, 256, 392)):
                    for kt in range(3):
                        nc.tensor.matmul(
                            pq[:, c0:c1].rearrange("p (w t) -> p w t", w=2),
                            lhsT=wqk[:, kt, 128 * fc:128 * (fc + 1)],
                            rhs=_mkap(xnT, hoff + 128 * kt, [[768, 2], [1, tw]]),
                            start=(kt == 0), stop=(kt == 2),
                            skip_group_check=True)
                # psum cols: [0:128) w0 lo, [128:256) w1 lo, [256:324) w0 hi,
                # [324:392) w1 hi -> qkT per-window contiguous tokens
                nc.vector.tensor_copy(
                    out=_mkap(qkT, 392 * fc, [[196, 2], [1, 128]]),
                    in_=pq[:, 0:256])
                nc.vector.tensor_copy(
                    out=_mkap(qkT, 392 * fc + 128, [[196, 2], [1, 68]]),
                    in_=pq[:, 256:392])

            # remap to per-head base-0 tiles [32, 24 slots, 392]:
            # slot = 4*chunk + j  (chunks 0-2 q, 3-5 k)
            qkT32 = qk32.tile([32, 24, 392], BF16, tag="qkT32")
            for j in range(4):
                nc.sync.dma_start(
                    out=_mkap(qkT32, 392 * j, [[4 * 392, 6], [1, 392]]),
                    in_=qkT[32 * j:32 * (j + 1), :, :])

            for wi in range(2):
                b = 2 * p + wi
                woff = 196 * wi

                # V projection into AV-ready layout
                va0 = vap.tile([128, 6, 2, 64], BF16, tag="va0")
                va1 = vap.tile([68, 6, 2, 64], BF16, tag="va1")
                for si, (va, rows) in enumerate(((va0, 128), (va1, 68))):
                    pv = psm.tile([128, 512], F32, tag="ps")
                    for kt in range(3):
                        nc.tensor.matmul(
                            pv[0:rows, 0:384],
                            lhsT=xnT[:, 2 * wi + si, kt, 0:rows],
                            rhs=wv[:, kt, :],
                            start=(kt == 0), stop=(kt == 2))
                    nc.vector.tensor_copy(
                        out=va[0:rows, :, :, 0:32],
                        in_=pv[0:rows, 0:384].rearrange(
                            "p (a b c) -> p a b c", a=6, b=2, c=32))
                    nc.vector.memset(va[:, :, :, 32:33], 1.0)

                # scores^T + exp + bias-mult (per tk-chunk)
                pbs = []
                for tkc, (tc0, trows) in enumerate(((0, 128), (128, 68))):
                    pss = pssp.tile([trows, 2352], F32, tag="pss")
                    for hh in range(12):
                        a0, a1 = 196 * hh, 196 * (hh + 1)
                        cuts = [a0] + [c for c in (512, 1024, 1536, 2048)
                                       if a0 < c < a1] + [a1]
                        for s0, s1 in zip(cuts[:-1], cuts[1:]):
                            nc.tensor.matmul(
                                pss[:, s0:s1],
                                lhsT=qkT32[0:32, 12 + hh,
                                           woff + tc0:woff + tc0 + trows],
                                rhs=qkT32[0:32, hh,
                                          woff + (s0 - a0):woff + (s1 - a0)],
                                start=True, stop=True)
                    eu = eup.tile([trows, 2352], BF16, tag=f"eu{tkc}")
                    nc.scalar.activation(out=eu, in_=pss, func=AF.Exp)
                    pb = pbp.tile([trows, 2352], BF16, tag=f"pb{tkc}")
                    ebt = eb0 if tkc == 0 else eb1
                    nc.gpsimd.tensor_tensor(
                        out=pb, in0=eu,
                        in1=ebt[0:trows].rearrange("p a b -> p (a b)"),
                        op=OP.mult)
                    pbs.append(pb)

                # AV with ones-column denominators; heads col-split at 0/64
                avU = avUs[wi]
                for ph in range(6):
                    pav = psm.tile([128, 512], F32, tag="ps")
                    for tkc, trows in ((0, 128), (1, 68)):
                        va = va0 if tkc == 0 else va1
                        for o in range(2):
                            h = 2 * ph + o
                            nc.tensor.matmul(
                                pav[64 * o:64 * o + 33, 0:196],
                                lhsT=va[0:trows, ph, o, 0:33],
                                rhs=pbs[tkc][:, 196 * h:196 * (h + 1)],
                                start=(tkc == 0), stop=(tkc == 1),
                                skip_group_check=True)
                    nc.vector.tensor_copy(out=avU[0:33, ph, 0:196],
                                          in_=pav[0:33, 0:196])
                    nc.vector.tensor_copy(out=avU[64:97, ph, 0:196],
                                          in_=pav[64:97, 0:196])

                # transpose to token-major [128, 12, 112]
                oaU = oaup.tile([128, 12, 112], BF16, tag="oaU")
                nc.sync.dma_start_transpose(
                    out=oaU, in_=avU.rearrange("p a b -> p (a b)"))

                rc = rcp.tile([128, 12, 2], F32, tag="rc")
                nc.vector.reciprocal(out=rc[:, :, 0], in_=oaU[:, :, 32])
                nc.vector.reciprocal(out=rc[:, :, 1], in_=oaU[:, :, 96])
                rcb = rcp.tile([128, 12, 2], BF16, tag="rcb")
                nc.vector.tensor_copy(out=rcb, in_=rc)

                # normalize into token-major [128(tq), 2, 384]
                oall = oap.tile([128, 2, 384], BF16, tag="oall")
                for c in range(2):
                    nc.vector.tensor_tensor(
                        out=oall[:, c, :].rearrange("p (a o u) -> p a o u",
                                                    a=6, o=2, u=32),
                        in0=_mkap(oaU, 112 * c, [[224, 6], [64, 2], [1, 32]]),
                        in1=_mkap(rcb, 2 * c, [[4, 6], [1, 2], [0, 32]]),
                        op=OP.mult)

                # transpose back to feature-major (dense blocks)
                outT = otp.tile([128, 2, 3, 128], BF16, tag="outT")
                nc.sync.dma_start_transpose(out=outT[:, 0], in_=oall[:, 0, :])
                nc.sync.dma_start_transpose(out=outT[:, 1], in_=oall[:, 1, :])

                # output projection
                for c, rows in ((0, 128), (1, 68)):
                    pp = psm.tile([128, 512], F32, tag="ps")
                    for kt in range(3):
                        nc.tensor.matmul(pp[0:rows, 0:384],
                                         lhsT=outT[:, c, kt, 0:rows],
                                         rhs=pw[:, kt, :],
                                         start=(kt == 0), stop=(kt == 2))
                    ob = obp.tile([128, 384], BF16, tag="ob")
                    nc.vector.tensor_copy(out=ob[0:rows], in_=pp[0:rows, 0:384])
                    nc.sync.dma_start(out=o_d[b, 128 * c:128 * c + rows, :],
                                      in_=ob[0:rows])

    nc.compile()
    return nc


def _host_prep(x, norm_w, norm_b, qkv_w, qkv_b, attention_biases, proj_w,
               proj_b, bias_idxs):
    """Layout/dtype transforms of inputs -> per-core in_maps."""
    Wp = (qkv_w * norm_w[None, :]).astype(np.float32)  # fold LN weight
    cvec = qkv_w.astype(np.float64) @ norm_b.astype(np.float64) + qkv_b
    assert np.allclose(cvec, 0.0, atol=1e-6), "nonzero qkv bias unsupported"
    assert np.allclose(proj_b, 0.0, atol=1e-6), "nonzero proj bias unsupported"

    # Wqk lhsT [384, 768]: chunks 0-2 q (pre-scaled), 3-5 k; head h at
    # chunk h//4, cols 32*(h%4).
    wqk = np.zeros((384, 768), np.float32)
    for h in range(H):
        fq = 128 * (h // 4) + 32 * (h % 4)
        fk = 384 + fq
        wqk[:, fq:fq + 32] = Wp[96 * h:96 * h + 32, :].T * SCALE
        wqk[:, fk:fk + 32] = Wp[96 * h + 32:96 * h + 64, :].T
    wqk = np.ascontiguousarray(
        np.transpose(wqk.reshape(3, 128, 768), (1, 0, 2))).astype(bf16)

    wvm = np.zeros((384, 384), np.float32)
    for h in range(H):
        wvm[:, 32 * h:32 * h + 32] = Wp[96 * h + 64:96 * h + 96, :].T
    wvm = np.ascontiguousarray(
        np.transpose(wvm.reshape(3, 128, 384), (1, 0, 2))).astype(bf16)

    pwm = np.ascontiguousarray(
        np.transpose(proj_w.T.reshape(3, 128, 384), (1, 0, 2))).astype(bf16)

    # expb[tk, h, tq] = exp(bias[h, tq, tk])
    ab = attention_biases[:, bias_idxs]  # [H, N, N] (tq, tk)
    expb = np.ascontiguousarray(
        np.transpose(np.exp(ab), (2, 0, 1))).astype(bf16)

    # LayerNorm fold on host + transpose to feature-major halves.
    mu = x.mean(axis=-1, keepdims=True, dtype=np.float32)
    xc = x - mu
    var = np.mean(xc * xc, axis=-1, keepdims=True, dtype=np.float32)
    xn = xc / np.sqrt(var + EPS)  # norm_w/b folded into W
    # [B, N, D] -> [128(dp), B, (s kt), 128(t)]
    xn_pad = np.zeros((B, 256, DIM), np.float32)
    xn_pad[:, :N, :] = xn
    xnt = xn_pad.reshape(B, 2, 128, 3, 128)          # [b, s, t, kt, dp]
    xnt = np.transpose(xnt, (4, 0, 1, 3, 2))         # [dp, b, s, kt, t]
    xnt = np.ascontiguousarray(xnt.reshape(128, B, 6, 128)).astype(bf16)

    xsh = xnt.reshape(128, NCORES, WPC, 6, 128)
    in_maps = []
    for i in range(NCORES):
        in_maps.append({
            "xn": np.ascontiguousarray(xsh[:, i]),
            "wqk": wqk, "wv": wvm, "pw": pwm, "expb": expb,
        })
    return in_maps


def _install_ntff_hook():
    """Wire the axon NTFF profiling hook this container ships but doesn't
    install (antenv.axon_hooks is absent from the image)."""
    import sys as _sys
    import types as _types
    try:
        from antenv.axon_hooks import get_axon_ntff_profile_hook  # noqa: F401
        return
    except ImportError:
        pass
    from trn_agent_boot.trn_boot import _ntff_profile_via_ctypes
    hook = _ntff_profile_via_ctypes("/opt/axon/libaxon_pjrt.so")
    mod = _types.ModuleType("antenv.axon_hooks")
    mod._hook = hook
    mod.get_axon_ntff_profile_hook = lambda: mod._hook
    mod.set_axon_ntff_profile_hook = lambda h: setattr(mod, "_hook", h)
    _sys.modules["antenv.axon_hooks"] = mod
    # keep artifacts local - no bucket access from this container
    import concourse.bass_utils as bu
    bu.upload_artifacts = lambda tmpdir: tmpdir


def kernel(x, norm_w, norm_b, qkv_w, qkv_b, attention_biases, proj_w, proj_b,
           bias_idxs, _trace=False):
    if _trace:
        _install_ntff_hook()
    x = np.asarray(x, np.float32)
    in_maps = _host_prep(
        x, np.asarray(norm_w, np.float32), np.asarray(norm_b, np.float32),
        np.asarray(qkv_w, np.float32), np.asarray(qkv_b, np.float32),
        np.asarray(attention_biases, np.float32),
        np.asarray(proj_w, np.float32), np.asarray(proj_b, np.float32),
        np.asarray(bias_idxs))

    if "nc" not in _CACHE:
        _CACHE["nc"] = _build_nc()
    nc = _CACHE["nc"]

    res = run_bass_kernel_spmd(nc, in_maps, core_ids=list(range(NCORES)),
                               trace=_trace)
    _CACHE["last_result"] = res
    out = np.concatenate(
        [np.asarray(r["out"]).astype(np.float32) for r in res.results], axis=0)
    return out.reshape(B, N, DIM)


# revision 14
# speedup vs baseline: 1.1324x; 1.1324x over previous
"""Windowed attention block (LeViT-style) on 8 TRN2 NeuronCores via Bass/Tile.

LayerNorm -> QKV -> per-head biased softmax attention -> output projection,
B=256 windows, N=196 tokens, DIM=384, 12 heads of dim 32.

Strategy: data-parallel over windows (32 per core), parameters replicated.
All matmuls in bf16 on the TensorEngine; softmax exp on ScalarE reading
scores straight out of PSUM; relative-position bias applied multiplicatively
(exp(s+b) = exp(s)*exp(b), exp(b) precomputed); softmax denominator comes
free from a ones-column appended to V; bias-mult on GPSIMD; transposes on
the DMA xbar (dense outputs only - strided xbar writes are broken on HW);
all matmul operands at partition base 0 (nonzero-base/tile_position packing
is broken on this stack), with a small SBUF->SBUF DMA remap giving per-head
base-0 q/k tiles.

Host-side prep: layout/dtype transforms (bf16 cast, weight reorder/transpose,
bias gather+exp) plus the input LayerNorm fold (stats+normalize+transpose),
which is <0.3% of the FLOPs.
"""

import numpy as np
import ml_dtypes

import concourse.bass as bass
import concourse.bacc as bacc
import concourse.tile as tile
from concourse import mybir
from concourse.bass_utils import run_bass_kernel_spmd

BF16 = mybir.dt.bfloat16
F32 = mybir.dt.float32
AF = mybir.ActivationFunctionType
OP = mybir.AluOpType
bf16 = ml_dtypes.bfloat16

B, N, DIM = 256, 196, 384
H, KD, VD = 12, 32, 32
EPS = 1e-5
NCORES = 8
WPC = B // NCORES  # windows per core
SCALE = KD ** -0.5

_CACHE = {}


def _mkap(t, extra_off, dims):
    """Manual AP: keep t's partition dim, custom free dims, +offset (elems)."""
    return bass.AP(
        tensor=t.tensor,
        offset=t.offset + extra_off,
        ap=[list(t.ap[0])] + [list(d) for d in dims],
    )


def _build_nc(nwin=WPC):
    nc = bacc.Bacc("TRN2", target_bir_lowering=False, debug=False)

    # xn: normalized input, pre-transposed to feature-major blocks.
    # [128, nwin, 6, 128]: window w, (s, kt) combined; element [dp, w, 3s+kt, t]
    # = xn[w, 128*s + t, 128*kt + dp]  (s=1 rows 68-127 are zero padding).
    xn_d = nc.dram_tensor("xn", [128, nwin, 6, 128], BF16, kind="ExternalInput")
    wqk_d = nc.dram_tensor("wqk", [128, 3, 768], BF16, kind="ExternalInput")
    wv_d = nc.dram_tensor("wv", [128, 3, 384], BF16, kind="ExternalInput")
    pw_d = nc.dram_tensor("pw", [128, 3, 384], BF16, kind="ExternalInput")
    eb_d = nc.dram_tensor("expb", [N, H, N], BF16, kind="ExternalInput")
    o_d = nc.dram_tensor("out", [nwin, N, DIM], BF16, kind="ExternalOutput")

    from contextlib import ExitStack

    with tile.TileContext(nc) as tc, ExitStack() as ctx:
        consts = ctx.enter_context(tc.tile_pool(name="consts", bufs=1))
        wqk = consts.tile([128, 3, 768], BF16, tag="wqk")
        nc.sync.dma_start(out=wqk, in_=wqk_d[:, :, :])
        wv = consts.tile([128, 3, 384], BF16, tag="wv")
        nc.sync.dma_start(out=wv, in_=wv_d[:, :, :])
        pw = consts.tile([128, 3, 384], BF16, tag="pw")
        nc.sync.dma_start(out=pw, in_=pw_d[:, :, :])
        eb0 = consts.tile([128, H, N], BF16, tag="eb0")
        nc.sync.dma_start(out=eb0, in_=eb_d[0:128, :, :])
        eb1 = consts.tile([68, H, N], BF16, tag="eb1")
        nc.sync.dma_start(out=eb1, in_=eb_d[128:196, :, :])

        # avU staging tiles live for the whole kernel so the pad region
        # (transposed but never consumed) is memset only once.
        avUs = [consts.tile([112, 6, 256], BF16, tag=f"avU{i}", name=f"avU{i}")
                for i in range(2)]
        for t in avUs:
            nc.vector.memset(t[:, :, :], 0.0)

        xnt = ctx.enter_context(tc.tile_pool(name="xnt", bufs=3))
        qkt = ctx.enter_context(tc.tile_pool(name="qkt", bufs=2))
        qk32 = ctx.enter_context(tc.tile_pool(name="qk32", bufs=2))
        vap = ctx.enter_context(tc.tile_pool(name="vap", bufs=2))
        eup = ctx.enter_context(tc.tile_pool(name="eup", bufs=2))
        pbp = ctx.enter_context(tc.tile_pool(name="pbp", bufs=2))
        oaup = ctx.enter_context(tc.tile_pool(name="oaup", bufs=2))
        oap = ctx.enter_context(tc.tile_pool(name="oap", bufs=2))
        otp = ctx.enter_context(tc.tile_pool(name="otp", bufs=2))
        obp = ctx.enter_context(tc.tile_pool(name="obp", bufs=4))
        rcp = ctx.enter_context(tc.tile_pool(name="rcp", bufs=2))

        pssp = ctx.enter_context(tc.tile_pool(name="pssp", bufs=2, space="PSUM"))
        psm = ctx.enter_context(tc.tile_pool(name="psm", bufs=2, space="PSUM"))

        for p in range(nwin // 2):
            # xnT per pair: [128, 4 halves (w0s0 w0s1 w1s0 w1s1), 3 kt, 128]
            xnT = xnt.tile([128, 4, 3, 128], BF16, tag="xnT")
            nc.sync.dma_start(
                out=xnT,
                in_=xn_d[:, 2 * p:2 * p + 2, :, :].rearrange(
                    "d w (s kt) t -> d (w s) kt t", s=2))

            qkT = qkt.tile([128, 6, 392], BF16, tag="qkT")
            # QK projection: rhs streams token halves of both windows.
            # Each psum col-region completes its kt accumulation before the
            # next region starts (start=True marks whole-bank zero regions).
            for fc in range(6):
                pq = psm.tile([128, 512], F32, tag="ps")
                for hoff, tw, c0, c1 in ((0, 128, 0, 256), (384, 68, 256, 392)):
                    for kt in range(3):
                        nc.tensor.matmul(
                            pq[:, c0:c1].rearrange("p (w t) -> p w t", w=2),
                            lhsT=wqk[:, kt, 128 * fc:128 * (fc + 1)],
                            rhs=_mkap(xnT, hoff + 128 * kt, [[768, 2], [1, tw]]),
                            start=(kt == 0), stop=(kt == 2),
                            skip_group_check=True)
                # psum cols: [0:128) w0 lo, [128:256) w1 lo, [256:324) w0 hi,
                # [324:392) w1 hi -> qkT per-window contiguous tokens
                nc.vector.tensor_copy(
                    out=_mkap(qkT, 392 * fc, [[196, 2], [1, 128]]),
                    in_=pq[:, 0:256])
                nc.vector.tensor_copy(
                    out=_mkap(qkT, 392 * fc + 128, [[196, 2], [1, 68]]),
                    in_=pq[:, 256:392])

            # remap to per-head base-0 tiles [32, 24 slots, 392]:
            # slot = 4*chunk + j  (chunks 0-2 q, 3-5 k)
            qkT32 = qk32.tile([32, 24, 392], BF16, tag="qkT32")
            for j in range(4):
                nc.sync.dma_start(
                    out=_mkap(qkT32, 392 * j, [[4 * 392, 6], [1, 392]]),
                    in_=qkT[32 * j:32 * (j + 1), :, :])

            for wi in range(2):
                b = 2 * p + wi
                woff = 196 * wi

                # V projection into AV-ready layout
                va0 = vap.tile([128, 6, 2, 64], BF16, tag="va0")
                va1 = vap.tile([68, 6, 2, 64], BF16, tag="va1")
                for si, (va, rows) in enumerate(((va0, 128), (va1, 68))):
                    pv = psm.tile([128, 512], F32, tag="ps")
                    for kt in range(3):
                        nc.tensor.matmul(
                            pv[0:rows, 0:384],
                            lhsT=xnT[:, 2 * wi + si, kt, 0:rows],
                            rhs=wv[:, kt, :],
                            start=(kt == 0), stop=(kt == 2))
                    nc.vector.tensor_copy(
                        out=va[0:rows, :, :, 0:32],
                        in_=pv[0:rows, 0:384].rearrange(
                            "p (a b c) -> p a b c", a=6, b=2, c=32))
                    nc.vector.memset(va[:, :, :, 32:33], 1.0)

                # scores^T + exp + bias-mult, per (tk-chunk, 6-head half)
                pbs = {}
                for tkc, (tc0, trows) in enumerate(((0, 128), (128, 68))):
                    ebt = eb0 if tkc == 0 else eb1
                    for hf in range(2):
                        pss = pssp.tile([trows, 1176], F32, tag="pss")
                        for hl in range(6):
                            hh = 6 * hf + hl
                            a0, a1 = 196 * hl, 196 * (hl + 1)
                            cuts = [a0] + [c for c in (512, 1024)
                                           if a0 < c < a1] + [a1]
                            for s0, s1 in zip(cuts[:-1], cuts[1:]):
                                nc.tensor.matmul(
                                    pss[:, s0:s1],
                                    lhsT=qkT32[0:32, 12 + hh,
                                               woff + tc0:woff + tc0 + trows],
                                    rhs=qkT32[0:32, hh,
                                              woff + (s0 - a0):woff + (s1 - a0)],
                                    start=True, stop=True)
                        eu = eup.tile([trows, 1176], BF16, tag=f"eu{tkc}{hf}")
                        nc.scalar.activation(out=eu, in_=pss, func=AF.Exp)
                        pb = pbp.tile([trows, 1176], BF16, tag=f"pb{tkc}{hf}")
                        nc.vector.tensor_tensor(
                            out=pb, in0=eu,
                            in1=ebt[0:trows, 6 * hf:6 * (hf + 1), :].rearrange(
                                "p a b -> p (a b)"),
                            op=OP.mult)
                        pbs[(tkc, hf)] = pb

                # AV with ones-column denominators; heads col-split at 0/64
                avU = avUs[wi]
                for ph in range(6):
                    pav = psm.tile([128, 512], F32, tag="ps")
                    for tkc, trows in ((0, 128), (1, 68)):
                        va = va0 if tkc == 0 else va1
                        for o in range(2):
                            h = 2 * ph + o
                            nc.tensor.matmul(
                                pav[64 * o:64 * o + 33, 0:196],
                                lhsT=va[0:trows, ph, o, 0:33],
                                rhs=pbs[(tkc, h // 6)][:, 196 * (h % 6):
                                                       196 * (h % 6) + 196],
                                start=(tkc == 0), stop=(tkc == 1),
                                skip_group_check=True)
                    nc.vector.tensor_copy(out=avU[0:33, ph, 0:196],
                                          in_=pav[0:33, 0:196])
                    nc.vector.tensor_copy(out=avU[64:97, ph, 0:196],
                                          in_=pav[64:97, 0:196])

                # transpose to token-major [128, 12, 112]
                oaU = oaup.tile([128, 12, 112], BF16, tag="oaU")
                nc.sync.dma_start_transpose(
                    out=oaU, in_=avU.rearrange("p a b -> p (a b)"))

                rc = rcp.tile([128, 12, 2], F32, tag="rc")
                nc.vector.reciprocal(out=rc[:, :, 0], in_=oaU[:, :, 32])
                nc.vector.reciprocal(out=rc[:, :, 1], in_=oaU[:, :, 96])
                rcb = rcp.tile([128, 12, 2], BF16, tag="rcb")
                nc.vector.tensor_copy(out=rcb, in_=rc)

                # normalize into token-major [128(tq), 2, 384]
                oall = oap.tile([128, 2, 384], BF16, tag="oall")
                for c in range(2):
                    nc.vector.tensor_tensor(
                        out=oall[:, c, :].rearrange("p (a o u) -> p a o u",
                                                    a=6, o=2, u=32),
                        in0=_mkap(oaU, 112 * c, [[224, 6], [64, 2], [1, 32]]),
                        in1=_mkap(rcb, 2 * c, [[4, 6], [1, 2], [0, 32]]),
                        op=OP.mult)

                # transpose back to feature-major (dense blocks)
                outT = otp.tile([128, 2, 3, 128], BF16, tag="outT")
                nc.sync.dma_start_transpose(out=outT[:, 0], in_=oall[:, 0, :])
                nc.sync.dma_start_transpose(out=outT[:, 1], in_=oall[:, 1, :])

                # output projection
                for c, rows in ((0, 128), (1, 68)):
                    pp = psm.tile([128, 512], F32, tag="ps")
                    for kt in range(3):
                        nc.tensor.matmul(pp[0:rows, 0:384],
                                         lhsT=outT[:, c, kt, 0:rows],
                                         rhs=pw[:, kt, :],
                                         start=(kt == 0), stop=(kt == 2))
                    ob = obp.tile([128, 384], BF16, tag="ob")
                    nc.vector.tensor_copy(out=ob[0:rows], in_=pp[0:rows, 0:384])
                    nc.sync.dma_start(out=o_d[b, 128 * c:128 * c + rows, :],
                                      in_=ob[0:rows])

    nc.compile()
    return nc


def _host_prep(x, norm_w, norm_b, qkv_w, qkv_b, attention_biases, proj_w,
               proj_b, bias_idxs):
    """Layout/dtype transforms of inputs -> per-core in_maps."""
    Wp = (qkv_w * norm_w[None, :]).astype(np.float32)  # fold LN weight
    cvec = qkv_w.astype(np.float64) @ norm_b.astype(np.float64) + qkv_b
    assert np.allclose(cvec, 0.0, atol=1e-6), "nonzero qkv bias unsupported"
    assert np.allclose(proj_b, 0.0, atol=1e-6), "nonzero proj bias unsupported"

    # Wqk lhsT [384, 768]: chunks 0-2 q (pre-scaled), 3-5 k; head h at
    # chunk h//4, cols 32*(h%4).
    wqk = np.zeros((384, 768), np.float32)
    for h in range(H):
        fq = 128 * (h // 4) + 32 * (h % 4)
        fk = 384 + fq
        wqk[:, fq:fq + 32] = Wp[96 * h:96 * h + 32, :].T * SCALE
        wqk[:, fk:fk + 32] = Wp[96 * h + 32:96 * h + 64, :].T
    wqk = np.ascontiguousarray(
        np.transpose(wqk.reshape(3, 128, 768), (1, 0, 2))).astype(bf16)

    wvm = np.zeros((384, 384), np.float32)
    for h in range(H):
        wvm[:, 32 * h:32 * h + 32] = Wp[96 * h + 64:96 * h + 96, :].T
    wvm = np.ascontiguousarray(
        np.transpose(wvm.reshape(3, 128, 384), (1, 0, 2))).astype(bf16)

    pwm = np.ascontiguousarray(
        np.transpose(proj_w.T.reshape(3, 128, 384), (1, 0, 2))).astype(bf16)

    # expb[tk, h, tq] = exp(bias[h, tq, tk])
    ab = attention_biases[:, bias_idxs]  # [H, N, N] (tq, tk)
    expb = np.ascontiguousarray(
        np.transpose(np.exp(ab), (2, 0, 1))).astype(bf16)

    # LayerNorm fold on host + transpose to feature-major halves.
    mu = x.mean(axis=-1, keepdims=True, dtype=np.float32)
    xc = x - mu
    var = np.mean(xc * xc, axis=-1, keepdims=True, dtype=np.float32)
    xn = xc / np.sqrt(var + EPS)  # norm_w/b folded into W
    # [B, N, D] -> [128(dp), B, (s kt), 128(t)]
    xn_pad = np.zeros((B, 256, DIM), np.float32)
    xn_pad[:, :N, :] = xn
    xnt = xn_pad.reshape(B, 2, 128, 3, 128)          # [b, s, t, kt, dp]
    xnt = np.transpose(xnt, (4, 0, 1, 3, 2))         # [dp, b, s, kt, t]
    xnt = np.ascontiguousarray(xnt.reshape(128, B, 6, 128)).astype(bf16)

    xsh = xnt.reshape(128, NCORES, WPC, 6, 128)
    in_maps = []
    for i in range(NCORES):
        in_maps.append({
            "xn": np.ascontiguousarray(xsh[:, i]),
            "wqk": wqk, "wv": wvm, "pw": pwm, "expb": expb,
        })
    return in_maps


def _install_ntff_hook():
    """Wire the axon NTFF profiling hook this container ships but doesn't
    install (antenv.axon_hooks is absent from the image)."""
    import sys as _sys
    import types as _types
    try:
        from antenv.axon_hooks import get_axon_ntff_profile_hook  # noqa: F401
        return
    except ImportError:
        pass
    from trn_agent_boot.trn_boot import _ntff_profile_via_ctypes
    hook = _ntff_profile_via_ctypes("/opt/axon/libaxon_pjrt.so")
    mod = _types.ModuleType("antenv.axon_hooks")
    mod._hook = hook
    mod.get_axon_ntff_profile_hook = lambda: mod._hook
    mod.set_axon_ntff_profile_hook = lambda h: setattr(mod, "_hook", h)
    _sys.modules["antenv.axon_hooks"] = mod
    # keep artifacts local - no bucket access from this container
    import concourse.bass_utils as bu
    bu.upload_artifacts = lambda tmpdir: tmpdir


def kernel(x, norm_w, norm_b, qkv_w, qkv_b, attention_biases, proj_w, proj_b,
           bias_idxs, _trace=False):
    if _trace:
        _install_ntff_hook()
    x = np.asarray(x, np.float32)
    in_maps = _host_prep(
        x, np.asarray(norm_w, np.float32), np.asarray(norm_b, np.float32),
        np.asarray(qkv_w, np.float32), np.asarray(qkv_b, np.float32),
        np.asarray(attention_biases, np.float32),
        np.asarray(proj_w, np.float32), np.asarray(proj_b, np.float32),
        np.asarray(bias_idxs))

    if "nc" not in _CACHE:
        _CACHE["nc"] = _build_nc()
    nc = _CACHE["nc"]

    res = run_bass_kernel_spmd(nc, in_maps, core_ids=list(range(NCORES)),
                               trace=_trace)
    _CACHE["last_result"] = res
    out = np.concatenate(
        [np.asarray(r["out"]).astype(np.float32) for r in res.results], axis=0)
    return out.reshape(B, N, DIM)


# revision 15
# speedup vs baseline: 1.5499x; 1.3688x over previous
"""Windowed attention block (LeViT-style) on 8 TRN2 NeuronCores via Bass/Tile.

LayerNorm -> QKV -> per-head biased softmax attention -> output projection,
B=256 windows, N=196 tokens, DIM=384, 12 heads of dim 32.

Strategy: data-parallel over windows (32 per core), parameters replicated.
All matmuls in bf16 on the TensorEngine; softmax exp on ScalarE reading
scores straight out of PSUM; relative-position bias applied multiplicatively
(exp(s+b) = exp(s)*exp(b), exp(b) precomputed); softmax denominator comes
free from a ones-column appended to V; bias-mult on GPSIMD; transposes on
the DMA xbar (dense outputs only - strided xbar writes are broken on HW);
all matmul operands at partition base 0 (nonzero-base/tile_position packing
is broken on this stack), with a small SBUF->SBUF DMA remap giving per-head
base-0 q/k tiles.

Host-side prep: layout/dtype transforms (bf16 cast, weight reorder/transpose,
bias gather+exp) plus the input LayerNorm fold (stats+normalize+transpose),
which is <0.3% of the FLOPs.
"""

import numpy as np
import ml_dtypes

import concourse.bass as bass
import concourse.bacc as bacc
import concourse.tile as tile
from concourse import mybir
from concourse.bass_utils import run_bass_kernel_spmd

BF16 = mybir.dt.bfloat16
F32 = mybir.dt.float32
AF = mybir.ActivationFunctionType
OP = mybir.AluOpType
bf16 = ml_dtypes.bfloat16

B, N, DIM = 256, 196, 384
H, KD, VD = 12, 32, 32
EPS = 1e-5
NCORES = 8
WPC = B // NCORES  # windows per core
SCALE = KD ** -0.5

_CACHE = {}


def _mkap(t, extra_off, dims):
    """Manual AP: keep t's partition dim, custom free dims, +offset (elems)."""
    return bass.AP(
        tensor=t.tensor,
        offset=t.offset + extra_off,
        ap=[list(t.ap[0])] + [list(d) for d in dims],
    )


def _build_nc(nwin=WPC, sim_safe=False):
    nc = bacc.Bacc("TRN2", target_bir_lowering=False, debug=False)

    # xn: normalized input, pre-transposed to feature-major blocks.
    # [128, nwin, 6, 128]: window w, (s, kt) combined; element [dp, w, 3s+kt, t]
    # = xn[w, 128*s + t, 128*kt + dp]  (s=1 rows 68-127 are zero padding).
    xn_d = nc.dram_tensor("xn", [128, nwin, 6, 128], BF16, kind="ExternalInput")
    wqk_d = nc.dram_tensor("wqk", [128, 3, 768], BF16, kind="ExternalInput")
    wv_d = nc.dram_tensor("wv", [128, 3, 384], BF16, kind="ExternalInput")
    pw_d = nc.dram_tensor("pw", [128, 3, 384], BF16, kind="ExternalInput")
    eb_d = nc.dram_tensor("expb", [N, H, N], BF16, kind="ExternalInput")
    o_d = nc.dram_tensor("out", [nwin, N, DIM], BF16, kind="ExternalOutput")

    from contextlib import ExitStack

    with tile.TileContext(nc) as tc, ExitStack() as ctx:
        consts = ctx.enter_context(tc.tile_pool(name="consts", bufs=1))
        wqk = consts.tile([128, 3, 768], BF16, tag="wqk")
        nc.sync.dma_start(out=wqk, in_=wqk_d[:, :, :])
        wv = consts.tile([128, 3, 384], BF16, tag="wv")
        nc.sync.dma_start(out=wv, in_=wv_d[:, :, :])
        pw = consts.tile([128, 3, 384], BF16, tag="pw")
        nc.sync.dma_start(out=pw, in_=pw_d[:, :, :])
        eb0 = consts.tile([128, H, N], BF16, tag="eb0")
        nc.sync.dma_start(out=eb0, in_=eb_d[0:128, :, :])
        eb1 = consts.tile([68, H, N], BF16, tag="eb1")
        nc.sync.dma_start(out=eb1, in_=eb_d[128:196, :, :])

        # avU staging tiles live for the whole kernel so the pad region
        # (transposed but never consumed) is memset only once.
        avUs = [consts.tile([112, 6, 256], BF16, tag=f"avU{i}", name=f"avU{i}")
                for i in range(2)]
        for t in avUs:
            nc.vector.memset(t[:, :, :], 0.0)

        xnt = ctx.enter_context(tc.tile_pool(name="xnt", bufs=3))
        qkt = ctx.enter_context(tc.tile_pool(name="qkt", bufs=2))
        qk32 = ctx.enter_context(tc.tile_pool(name="qk32", bufs=2))
        vap = ctx.enter_context(tc.tile_pool(name="vap", bufs=2))
        eup = ctx.enter_context(tc.tile_pool(name="eup", bufs=2))
        pbp = ctx.enter_context(tc.tile_pool(name="pbp", bufs=2))
        oaup = ctx.enter_context(tc.tile_pool(name="oaup", bufs=2))
        oap = ctx.enter_context(tc.tile_pool(name="oap", bufs=2))
        otp = ctx.enter_context(tc.tile_pool(name="otp", bufs=3))
        obp = ctx.enter_context(tc.tile_pool(name="obp", bufs=4))
        rcp = ctx.enter_context(tc.tile_pool(name="rcp", bufs=2))

        pssp = ctx.enter_context(tc.tile_pool(name="pssp", bufs=2, space="PSUM"))
        psm = ctx.enter_context(tc.tile_pool(name="psm", bufs=2, space="PSUM"))

        pending_proj = []

        def emit_proj():
            outT, b = pending_proj.pop(0)
            for c, rows in ((0, 128), (1, 68)):
                pp = psm.tile([128, 512], F32, tag="ps", name="pp")
                for kt in range(3):
                    nc.tensor.matmul(pp[0:rows, 0:384],
                                     lhsT=outT[:, c, kt, 0:rows],
                                     rhs=pw[:, kt, :],
                                     start=(kt == 0), stop=(kt == 2))
                ob = obp.tile([128, 384], BF16, tag="ob", name="ob")
                nc.scalar.activation(out=ob[0:rows], in_=pp[0:rows, 0:384],
                                     func=AF.Copy)
                nc.sync.dma_start(out=o_d[b, 128 * c:128 * c + rows, :],
                                  in_=ob[0:rows])

        for p in range(nwin // 2):
            # xnT per pair: [128, 4 halves (w0s0 w0s1 w1s0 w1s1), 3 kt, 128]
            xnT = xnt.tile([128, 4, 3, 128], BF16, tag="xnT")
            nc.sync.dma_start(
                out=xnT,
                in_=xn_d[:, 2 * p:2 * p + 2, :, :].rearrange(
                    "d w (s kt) t -> d (w s) kt t", s=2))

            qkT = qkt.tile([128, 6, 392], BF16, tag="qkT")
            # QK projection: rhs streams token halves of both windows.
            # Each psum col-region completes its kt accumulation before the
            # next region starts (start=True marks whole-bank zero regions).
            for fc in range(6):
                pq = psm.tile([128, 512], F32, tag="ps")
                for hoff, tw, c0, c1 in ((0, 128, 0, 256), (384, 68, 256, 392)):
                    for kt in range(3):
                        nc.tensor.matmul(
                            pq[:, c0:c1].rearrange("p (w t) -> p w t", w=2),
                            lhsT=wqk[:, kt, 128 * fc:128 * (fc + 1)],
                            rhs=_mkap(xnT, hoff + 128 * kt, [[768, 2], [1, tw]]),
                            start=(kt == 0), stop=(kt == 2),
                            skip_group_check=True)
                # psum cols: [0:128) w0 lo, [128:256) w1 lo, [256:324) w0 hi,
                # [324:392) w1 hi -> qkT per-window contiguous tokens
                nc.vector.tensor_copy(
                    out=_mkap(qkT, 392 * fc, [[196, 2], [1, 128]]),
                    in_=pq[:, 0:256])
                nc.vector.tensor_copy(
                    out=_mkap(qkT, 392 * fc + 128, [[196, 2], [1, 68]]),
                    in_=pq[:, 256:392])

            # remap to per-head base-0 tiles [32, 24 slots, 392]:
            # slot = 4*chunk + j  (chunks 0-2 q, 3-5 k)
            qkT32 = qk32.tile([32, 24, 392], BF16, tag="qkT32")
            for j in range(4):
                nc.sync.dma_start(
                    out=_mkap(qkT32, 392 * j, [[4 * 392, 6], [1, 392]]),
                    in_=qkT[32 * j:32 * (j + 1), :, :])

            for wi in range(2):
                b = 2 * p + wi
                woff = 196 * wi

                # V projection into AV-ready layout
                va0 = vap.tile([128, 6, 2, 64], BF16, tag="va0")
                va1 = vap.tile([68, 6, 2, 64], BF16, tag="va1")
                for si, (va, rows) in enumerate(((va0, 128), (va1, 68))):
                    pv = psm.tile([128, 512], F32, tag="ps")
                    for kt in range(3):
                        nc.tensor.matmul(
                            pv[0:rows, 0:384],
                            lhsT=xnT[:, 2 * wi + si, kt, 0:rows],
                            rhs=wv[:, kt, :],
                            start=(kt == 0), stop=(kt == 2))
                    nc.vector.tensor_copy(
                        out=va[0:rows, :, :, 0:32],
                        in_=pv[0:rows, 0:384].rearrange(
                            "p (a b c) -> p a b c", a=6, b=2, c=32))
                    nc.vector.memset(va[:, :, :, 32:33], 1.0)

                # scores^T + exp + bias-mult, per (tk-chunk, 6-head half)
                pbs = {}
                for tkc, (tc0, trows) in enumerate(((0, 128), (128, 68))):
                    ebt = eb0 if tkc == 0 else eb1
                    for hf in range(2):
                        pss = pssp.tile([trows, 1176], F32, tag="pss")
                        for hl in range(6):
                            hh = 6 * hf + hl
                            a0, a1 = 196 * hl, 196 * (hl + 1)
                            cuts = [a0] + [c for c in (512, 1024)
                                           if a0 < c < a1] + [a1]
                            for s0, s1 in zip(cuts[:-1], cuts[1:]):
                                nc.tensor.matmul(
                                    pss[:, s0:s1],
                                    lhsT=qkT32[0:32, 12 + hh,
                                               woff + tc0:woff + tc0 + trows],
                                    rhs=qkT32[0:32, hh,
                                              woff + (s0 - a0):woff + (s1 - a0)],
                                    start=True, stop=True)
                        eu = eup.tile([trows, 1176], BF16, tag=f"eu{tkc}{hf}")
                        nc.scalar.activation(out=eu, in_=pss, func=AF.Exp)
                        pb = pbp.tile([trows, 1176], BF16, tag=f"pb{tkc}{hf}")
                        nc.vector.tensor_tensor(
                            out=pb, in0=eu,
                            in1=ebt[0:trows, 6 * hf:6 * (hf + 1), :].rearrange(
                                "p a b -> p (a b)"),
                            op=OP.mult)
                        pbs[(tkc, hf)] = pb

                # previous window's projection slots into the gap while
                # this window's exp/bias-mult chain completes
                if len(pending_proj) > 1:
                    emit_proj()

                # AV with ones-column denominators; heads col-split at 0/64
                avU = avUs[wi]
                for ph in range(6):
                    pav = psm.tile([128, 512], F32, tag="ps")
                    for tkc, trows in ((0, 128), (1, 68)):
                        va = va0 if tkc == 0 else va1
                        for o in range(2):
                            h = 2 * ph + o
                            nc.tensor.matmul(
                                pav[64 * o:64 * o + 33, 0:196],
                                lhsT=va[0:trows, ph, o, 0:33],
                                rhs=pbs[(tkc, h // 6)][:, 196 * (h % 6):
                                                       196 * (h % 6) + 196],
                                start=(tkc == 0), stop=(tkc == 1),
                                skip_group_check=True)
                    if sim_safe:
                        nc.vector.tensor_copy(out=avU[0:33, ph, 0:196],
                                              in_=pav[0:33, 0:196])
                        nc.vector.tensor_copy(out=avU[64:97, ph, 0:196],
                                              in_=pav[64:97, 0:196])
                    else:
                        # rows 33-63/97-111 are stale psum bytes; they land in
                        # avU columns no downstream AP ever reads
                        nc.vector.tensor_copy(out=avU[0:112, ph, 0:196],
                                              in_=pav[0:112, 0:196])

                # transpose to token-major [128, 12, 112]
                oaU = oaup.tile([128, 12, 112], BF16, tag="oaU")
                nc.sync.dma_start_transpose(
                    out=oaU, in_=avU.rearrange("p a b -> p (a b)"))

                rc = rcp.tile([128, 12, 2], F32, tag="rc")
                nc.vector.reciprocal(out=rc[:, :, 0], in_=oaU[:, :, 32])
                nc.vector.reciprocal(out=rc[:, :, 1], in_=oaU[:, :, 96])
                rcb = rcp.tile([128, 12, 2], BF16, tag="rcb")
                nc.vector.tensor_copy(out=rcb, in_=rc)

                # normalize into token-major [128(tq), 2, 384]
                oall = oap.tile([128, 2, 384], BF16, tag="oall")
                for c in range(2):
                    nc.vector.tensor_tensor(
                        out=oall[:, c, :].rearrange("p (a o u) -> p a o u",
                                                    a=6, o=2, u=32),
                        in0=_mkap(oaU, 112 * c, [[224, 6], [64, 2], [1, 32]]),
                        in1=_mkap(rcb, 2 * c, [[4, 6], [1, 2], [0, 32]]),
                        op=OP.mult)

                # transpose back to feature-major (dense blocks)
                outT = otp.tile([128, 2, 3, 128], BF16, tag="outT")
                nc.sync.dma_start_transpose(out=outT[:, 0], in_=oall[:, 0, :])
                nc.sync.dma_start_transpose(out=outT[:, 1], in_=oall[:, 1, :])
                pending_proj.append((outT, b))

            pass
        while pending_proj:
            emit_proj()

    nc.compile()
    return nc


def _host_prep(x, norm_w, norm_b, qkv_w, qkv_b, attention_biases, proj_w,
               proj_b, bias_idxs):
    """Layout/dtype transforms of inputs -> per-core in_maps."""
    Wp = (qkv_w * norm_w[None, :]).astype(np.float32)  # fold LN weight
    cvec = qkv_w.astype(np.float64) @ norm_b.astype(np.float64) + qkv_b
    assert np.allclose(cvec, 0.0, atol=1e-6), "nonzero qkv bias unsupported"
    assert np.allclose(proj_b, 0.0, atol=1e-6), "nonzero proj bias unsupported"

    # Wqk lhsT [384, 768]: chunks 0-2 q (pre-scaled), 3-5 k; head h at
    # chunk h//4, cols 32*(h%4).
    wqk = np.zeros((384, 768), np.float32)
    for h in range(H):
        fq = 128 * (h // 4) + 32 * (h % 4)
        fk = 384 + fq
        wqk[:, fq:fq + 32] = Wp[96 * h:96 * h + 32, :].T * SCALE
        wqk[:, fk:fk + 32] = Wp[96 * h + 32:96 * h + 64, :].T
    wqk = np.ascontiguousarray(
        np.transpose(wqk.reshape(3, 128, 768), (1, 0, 2))).astype(bf16)

    wvm = np.zeros((384, 384), np.float32)
    for h in range(H):
        wvm[:, 32 * h:32 * h + 32] = Wp[96 * h + 64:96 * h + 96, :].T
    wvm = np.ascontiguousarray(
        np.transpose(wvm.reshape(3, 128, 384), (1, 0, 2))).astype(bf16)

    pwm = np.ascontiguousarray(
        np.transpose(proj_w.T.reshape(3, 128, 384), (1, 0, 2))).astype(bf16)

    # expb[tk, h, tq] = exp(bias[h, tq, tk])
    ab = attention_biases[:, bias_idxs]  # [H, N, N] (tq, tk)
    expb = np.ascontiguousarray(
        np.transpose(np.exp(ab), (2, 0, 1))).astype(bf16)

    # LayerNorm fold on host + transpose to feature-major halves.
    mu = x.mean(axis=-1, keepdims=True, dtype=np.float32)
    xc = x - mu
    var = np.mean(xc * xc, axis=-1, keepdims=True, dtype=np.float32)
    xn = xc / np.sqrt(var + EPS)  # norm_w/b folded into W
    # [B, N, D] -> [128(dp), B, (s kt), 128(t)]
    xn_pad = np.zeros((B, 256, DIM), np.float32)
    xn_pad[:, :N, :] = xn
    xnt = xn_pad.reshape(B, 2, 128, 3, 128)          # [b, s, t, kt, dp]
    xnt = np.transpose(xnt, (4, 0, 1, 3, 2))         # [dp, b, s, kt, t]
    xnt = np.ascontiguousarray(xnt.reshape(128, B, 6, 128)).astype(bf16)

    xsh = xnt.reshape(128, NCORES, WPC, 6, 128)
    in_maps = []
    for i in range(NCORES):
        in_maps.append({
            "xn": np.ascontiguousarray(xsh[:, i]),
            "wqk": wqk, "wv": wvm, "pw": pwm, "expb": expb,
        })
    return in_maps


def _install_ntff_hook():
    """Wire the axon NTFF profiling hook this container ships but doesn't
    install (antenv.axon_hooks is absent from the image)."""
    import sys as _sys
    import types as _types
    try:
        from antenv.axon_hooks import get_axon_ntff_profile_hook  # noqa: F401
        return
    except ImportError:
        pass
    from trn_agent_boot.trn_boot import _ntff_profile_via_ctypes
    hook = _ntff_profile_via_ctypes("/opt/axon/libaxon_pjrt.so")
    mod = _types.ModuleType("antenv.axon_hooks")
    mod._hook = hook
    mod.get_axon_ntff_profile_hook = lambda: mod._hook
    mod.set_axon_ntff_profile_hook = lambda h: setattr(mod, "_hook", h)
    _sys.modules["antenv.axon_hooks"] = mod
    # keep artifacts local - no bucket access from this container
    import concourse.bass_utils as bu
    bu.upload_artifacts = lambda tmpdir: tmpdir


def kernel(x, norm_w, norm_b, qkv_w, qkv_b, attention_biases, proj_w, proj_b,
           bias_idxs, _trace=False):
    if _trace:
        _install_ntff_hook()
    x = np.asarray(x, np.float32)
    in_maps = _host_prep(
        x, np.asarray(norm_w, np.float32), np.asarray(norm_b, np.float32),
        np.asarray(qkv_w, np.float32), np.asarray(qkv_b, np.float32),
        np.asarray(attention_biases, np.float32),
        np.asarray(proj_w, np.float32), np.asarray(proj_b, np.float32),
        np.asarray(bias_idxs))

    if "nc" not in _CACHE:
        _CACHE["nc"] = _build_nc()
    nc = _CACHE["nc"]

    res = run_bass_kernel_spmd(nc, in_maps, core_ids=list(range(NCORES)),
                               trace=_trace)
    _CACHE["last_result"] = res
    out = np.concatenate(
        [np.asarray(r["out"]).astype(np.float32) for r in res.results], axis=0)
    return out.reshape(B, N, DIM)
